# revision 35
# baseline (speedup 1.0000x reference)
"""CvT attention block (depthwise conv QKV + MHA) on 8 Trainium2 NeuronCores,
data-parallel over batch; the whole per-image computation repeats inside an
on-device For_i loop so amortized timing reflects steady-state HW execution.

Software-pipelined loop body (2 images per iteration, ping-pong buffers):
phase C (attention) for image buffer b runs with phases A/B (convs +
projections) for buffer 1-b interleaved into its l-chunk loop, so the
DVE-heavy convs overlap the PE/ACT-heavy attention.

  A) depthwise 3x3 convs on DVE in fp16 at 2x rate: the host pre-shifts the
     image (center/left/right columns for the stride-1 conv; six parity-
     compacted planes for the stride-2 convs) so every tap is a full-width
     unit-stride aligned scalar_tensor_tensor FMA.
  B) projections with fp16 operands (f32 PSUM): Q^T/K^T in [co, l] layout,
     V-hat in [t, co] layout with a ones column per head (softmax denominators
     fall out of the AV matmul for free).
  C) per (l-chunk of 512, head): S^T = K_h Q_h^T via PE into 2-bank PSUM
     tiles (bufs=3 so scores of head h+1 overlap Exp of head h), Exp on ACT
     (fp16 out), AV accumulation in f32 PSUM; then reciprocal +
     indicator-matmul broadcast for the softmax normalization, and the output
     projection in [l, co] layout with a transposing DMA store.
"""

import contextlib
import numpy as np
import ml_dtypes
from concourse import mybir
import concourse.bacc as bacc
import concourse.tile as tile
from concourse.bass_utils import run_bass_kernel_spmd

F32 = mybir.dt.float32
F16 = mybir.dt.float16
AFT = mybir.ActivationFunctionType
ALU = mybir.AluOpType

C = 384
T = 3136            # 56*56
TKV = 784           # 28*28
NH = 6
SCALE = C ** (-0.5)
EPS = 1e-5
XP = 56 + T + 56    # 3248: row r, col c of the image at flat 56 + r*56 + c
PLW = 30 * 28       # 840: one parity-compacted plane (28 cols x 30 rows)
XIN = 3 * XP + 6 * PLW   # 14784 fp16 per partition

LC = [(i * 512, min(512, T - i * 512)) for i in range(7)]
T_TILES = [(i * 128, min(128, TKV - i * 128)) for i in range(7)]
TGRP = [T_TILES[0:2], T_TILES[2:4], T_TILES[4:6], T_TILES[6:7]]

_CACHE = {}


def _conv_q(nc, xin, yq, wb):
    """Stride-1 depthwise 3x3 conv; all taps full-width aligned fp16 FMAs.
    xin columns: [xc | xl | xr | planes...]; xl/xr are column-shifted copies
    (xl[p] = x[r, c-1], xr[p] = x[r, c+1], zero at the seam)."""
    w = lambda t: wb[:, t:t + 1]
    base = {-1: XP, 0: 0, 1: 2 * XP}
    nc.vector.tensor_scalar(yq[:], xin[:, 56:56 + T], w(4), wb[:, 27:28],
                            op0=ALU.mult, op1=ALU.add)
    for t in (0, 1, 2, 3, 5, 6, 7, 8):
        di, dj = t // 3 - 1, t % 3 - 1
        off = base[dj] + 56 + 56 * di
        nc.vector.scalar_tensor_tensor(yq[:], xin[:, off:off + T], w(t),
                                       yq[:], op0=ALU.mult, op1=ALU.add)


def _conv_kv(nc, xin, yo, wb, cv):
    """Stride-2 depthwise 3x3 conv from parity-compacted planes.
    Plane (parity p, column-shift dj) at 3*XP + (3p + dj + 1)*PLW, laid out
    as 30 rows x 28 cols; row j=1+k holds x[2k + p] columns (2c + dj)."""
    w = lambda t: wb[:, 9 * cv + t:9 * cv + t + 1]
    bias = wb[:, 27 + cv:28 + cv]

    def tap(di, dj):
        par = 0 if di == 0 else 1
        j0 = 0 if di == -1 else 1
        return 3 * XP + (3 * par + dj + 1) * PLW + 28 * j0

    o = tap(0, 0)
    nc.vector.tensor_scalar(yo[:], xin[:, o:o + TKV], w(4), bias,
                            op0=ALU.mult, op1=ALU.add)
    for t in (0, 1, 2, 3, 5, 6, 7, 8):
        di, dj = t // 3 - 1, t % 3 - 1
        o = tap(di, dj)
        nc.vector.scalar_tensor_tensor(yo[:], xin[:, o:o + TKV], w(t),
                                       yo[:], op0=ALU.mult, op1=ALU.add)


def _emit(nc, tc, ctx, d, reps, loop=True):
    pers = ctx.enter_context(tc.tile_pool(name="pers", bufs=1))

    wq = [pers.tile([128, C], F16, tag=f"wq{i}", name=f"wq{i}") for i in range(3)]
    wk = [pers.tile([128, C], F16, tag=f"wk{i}", name=f"wk{i}") for i in range(3)]
    wvp = [pers.tile([128, NH * 65], F16, tag=f"wvp{i}", name=f"wvp{i}")
           for i in range(3)]
    wpj = [pers.tile([128, C], F16, tag=f"wpj{i}", name=f"wpj{i}")
           for i in range(3)]
    ind6 = [pers.tile([6, 128], F16, tag=f"ind6{i}", name=f"ind6{i}")
            for i in range(3)]
    wb = [pers.tile([128, 30], F32, tag=f"wb{i}", name=f"wb{i}")
          for i in range(3)]
    bpjW = pers.tile([128, 512], F32, tag="bpjW", name="bpjW")
    # double-buffered intermediates (ping-pong between the two body halves)
    QT = [[pers.tile([128, T], F16, tag=f"QT{b}{i}", name=f"QT{b}{i}")
           for i in range(3)] for b in range(2)]
    KT = [[pers.tile([128, TKV], F16, tag=f"KT{b}{i}", name=f"KT{b}{i}")
           for i in range(3)] for b in range(2)]
    Vh = [[pers.tile([128, NH * 65], F16, tag=f"Vh{b}{i}", name=f"Vh{b}{i}")
           for i in range(7)] for b in range(2)]

    for i in range(3):
        nc.sync.dma_start(wq[i][:], d["wq"][i * 128:(i + 1) * 128, :])
        nc.sync.dma_start(wk[i][:], d["wk"][i * 128:(i + 1) * 128, :])
        nc.sync.dma_start(wvp[i][:], d["wvp"][i * 128:(i + 1) * 128, :])
        nc.sync.dma_start(wpj[i][:], d["wpj"][i * 128:(i + 1) * 128, :])
        nc.sync.dma_start(ind6[i][:], d["ind6"][i])
        nc.sync.dma_start(wb[i][:], d["wb"][i])
    nc.sync.dma_start(bpjW[:], d["bpjW"])

    def proj_pieces(pool, tag, yq, yk, yv, ws, which):
        """Projection work writing buffer set ws. which: 0-2 = Q co, 3 = K+V."""
        if which < 3:
            co = which
            for w in range(4):
                chunks = LC[2 * w:2 * w + 2]
                p = pool.tile([128, 1024], F32, tag=tag, name=tag)
                for k, (lo, ls) in enumerate(chunks):
                    for ch in range(3):
                        nc.tensor.matmul(
                            p[0:128, k * 512:k * 512 + ls],
                            wq[ch][:, co * 128:(co + 1) * 128],
                            yq[ch][:, lo:lo + ls],
                            start=(ch == 0), stop=(ch == 2))
                base = chunks[0][0]
                wid = sum(c[1] for c in chunks)
                nc.vector.tensor_copy(QT[ws][co][:, base:base + wid],
                                      p[:, 0:wid])
        else:
            for co in range(3):
                p = pool.tile([128, 1024], F32, tag=tag, name=tag)
                for k, (to, ts) in enumerate(((0, 512), (512, 272))):
                    for ch in range(3):
                        nc.tensor.matmul(
                            p[:, k * 512:k * 512 + ts],
                            wk[ch][:, co * 128:(co + 1) * 128],
                            yk[ch][:, to:to + ts],
                            start=(ch == 0), stop=(ch == 2))
                nc.vector.tensor_copy(KT[ws][co][:], p[:, 0:TKV])
            for gi in range(4):
                tt = T_TILES[2 * gi:2 * gi + 2]
                p = pool.tile([128, 1024], F32, tag=tag, name=tag)
                for k, (to, ts) in enumerate(tt):
                    for ch in range(3):
                        nc.tensor.matmul(
                            p[0:ts, k * 512:k * 512 + NH * 65],
                            yv[ch][:, to:to + ts], wvp[ch][:],
                            start=(ch == 0), stop=(ch == 2))
                for k, (to, ts) in enumerate(tt):
                    ti = 2 * gi + k
                    nc.vector.tensor_copy(
                        Vh[ws][ti][0:ts, :],
                        p[0:ts, k * 512:k * 512 + NH * 65])
                    nc.vector.memset(Vh[ws][ti][0:ts, 64:NH * 65:65], 1.0)

    def emit_AB_full(pfx, ws):
        """Standalone conv + projection phases (prologue only)."""
        with contextlib.ExitStack() as phAB:
            ypool = phAB.enter_context(tc.tile_pool(name="y" + pfx, bufs=1))
            yq = [ypool.tile([128, T], F16, tag=f"yq{i}", name=f"yq{i}")
                  for i in range(3)]
            yk = [ypool.tile([128, TKV], F16, tag=f"yk{i}", name=f"yk{i}")
                  for i in range(3)]
            yv = [ypool.tile([128, TKV], F16, tag=f"yv{i}", name=f"yv{i}")
                  for i in range(3)]
            with contextlib.ExitStack() as phA:
                xpool = phA.enter_context(tc.tile_pool(name="x" + pfx, bufs=2))
                for ch in range(3):
                    xt = xpool.tile([128, XIN], F16, tag="x", name="x")
                    nc.sync.dma_start(xt[:], d["xin"][ch * 128:(ch + 1) * 128, :])
                    _conv_q(nc, xt, yq[ch], wb[ch])
                    _conv_kv(nc, xt, yk[ch], wb[ch], 1)
                    _conv_kv(nc, xt, yv[ch], wb[ch], 2)
            with contextlib.ExitStack() as phB:
                psB = phB.enter_context(
                    tc.tile_pool(name="psB" + pfx, bufs=2, space="PSUM"))
                for which in range(4):
                    proj_pieces(psB, "psB", yq, yk, yv, ws, which)

    def emit_CAB(cfx, rs, ws):
        """Phase C on buffer set rs with phases A/B for set ws interleaved
        into the l-chunk loop."""
        with contextlib.ExitStack() as phC:
            cw = phC.enter_context(tc.tile_pool(name="cw" + cfx, bufs=2))
            psS = phC.enter_context(
                tc.tile_pool(name="psS" + cfx, bufs=3, space="PSUM"))
            psO = phC.enter_context(
                tc.tile_pool(name="psO" + cfx, bufs=2, space="PSUM"))
            ypool = phC.enter_context(tc.tile_pool(name="y" + cfx, bufs=1))
            xpool = phC.enter_context(tc.tile_pool(name="xp" + cfx, bufs=2))
            yq = [ypool.tile([128, T], F16, tag=f"yq{i}", name=f"yq{i}")
                  for i in range(3)]
            yk = [ypool.tile([128, TKV], F16, tag=f"yk{i}", name=f"yk{i}")
                  for i in range(3)]
            yv = [ypool.tile([128, TKV], F16, tag=f"yv{i}", name=f"yv{i}")
                  for i in range(3)]

            def ab_piece(i):
                if i < 3:
                    ch = i
                    xt = xpool.tile([128, XIN], F16, tag="x", name="x")
                    nc.sync.dma_start(xt[:], d["xin"][ch * 128:(ch + 1) * 128, :])
                    _conv_q(nc, xt, yq[ch], wb[ch])
                    _conv_kv(nc, xt, yk[ch], wb[ch], 1)
                    _conv_kv(nc, xt, yv[ch], wb[ch], 2)
                elif i < 7:
                    proj_pieces(psS, "psS", yq, yk, yv, ws, i - 3)

            for ci, (lo, ls) in enumerate(LC):
                OTb = cw.tile([128, 1536], F16, tag="otb", name="otb")
                rcf = cw.tile([1, NH * 512], F16, tag="rcf", name="rcf")
                rc6 = cw.tile([6, 512], F16, tag="rc6", name="rc6")
                rc6r = cw.tile([6, 512], F16, tag="rc6r", name="rc6r")
                for h in range(NH):
                    c2, po = h // 2, 64 * (h % 2)
                    ets = []
                    for gi, tt in enumerate(TGRP):
                        p = psS.tile([128, 1024], F32, tag="psS", name="psS")
                        for k, (to, ts) in enumerate(tt):
                            nc.tensor.matmul(p[0:ts, k * 512:k * 512 + ls],
                                             KT[rs][c2][po:po + 64, to:to + ts],
                                             QT[rs][c2][po:po + 64, lo:lo + ls],
                                             start=True, stop=True)
                        et = cw.tile([128, 1024], F16, tag="et", name="et",
                                     bufs=6)
                        wid = (len(tt) - 1) * 512 + ls
                        nc.scalar.activation(et[:, 0:wid], p[:, 0:wid],
                                             AFT.Exp, scale=float(SCALE))
                        ets.append(et)
                    po2 = psO.tile([128, 512], F32, tag="psO", name="psO")
                    for ti, (to, ts) in enumerate(T_TILES):
                        nc.tensor.matmul(
                            po2[0:65, :ls],
                            Vh[rs][ti][0:ts, h * 65:(h + 1) * 65],
                            ets[ti // 2][0:ts, (ti % 2) * 512:(ti % 2) * 512 + ls],
                            start=(ti == 0), stop=(ti == 6))
                    nc.vector.tensor_copy(
                        OTb[po:po + 64, c2 * 512:c2 * 512 + ls], po2[0:64, :ls])
                    nc.vector.tensor_copy(rcf[0:1, h * 512:h * 512 + ls],
                                          po2[64:65, :ls])
                # gather sums -> 6 partitions, recip, indicator bcast, scale
                rin = rcf[0:1, :].rearrange("p (g l) -> p g l", l=512)
                nc.sync.dma_start(rc6[0:6, 0:ls], rin[:, :, 0:ls])
                with nc.allow_low_precision(reason="fp16 recip"):
                    nc.vector.reciprocal(rc6r[0:6, 0:ls], rc6[0:6, 0:ls])
                rbps = [psO.tile([128, 512], F32, tag="psO", name="psO")
                        for _ in range(3)]
                for ch in range(3):
                    nc.tensor.matmul(rbps[ch][:, 0:ls], ind6[ch][:],
                                     rc6r[0:6, 0:ls], start=True, stop=True)
                for ch in range(3):
                    nc.vector.tensor_mul(
                        OTb[:, ch * 512:ch * 512 + ls],
                        OTb[:, ch * 512:ch * 512 + ls], rbps[ch][:, 0:ls])
                # output projection: [l, co] layout, one psum tile per l-128
                osb = cw.tile([128, 2048], F32, tag="o", name="o")
                nlt = (ls + 127) // 128
                for k in range(nlt):
                    lsz = min(128, ls - k * 128)
                    p = psO.tile([128, 512], F32, tag="psO", name="psO")
                    win = p[0:lsz, 0:C]
                    for ch in range(3):
                        nc.tensor.matmul(
                            win, OTb[:, ch * 512 + k * 128:
                                     ch * 512 + k * 128 + lsz],
                            wpj[ch][:], start=(ch == 0), stop=(ch == 2))
                    nc.vector.tensor_add(osb[0:lsz, k * 512:k * 512 + C],
                                         win, bpjW[0:lsz, 0:C])
                ov = osb[:].rearrange("p (w c) -> p w c", c=512)[:, 0:nlt, 0:C]
                lsz = min(128, ls - (nlt - 1) * 128)
                if lsz < 128:
                    ov = ov[0:lsz]
                dst = d["out"][lo:lo + ls, :].rearrange(
                    "(w p) c -> p w c", p=min(128, ls))
                nc.sync.dma_start(dst, ov)
                # interleave next image's convs / projections
                ab_piece(ci)

    # Software pipeline: prologue fills buffer set 0; each loop iteration
    # processes two images (C on one set while A/B refill the other).
    emit_AB_full("p", 0)
    if loop:
        with tc.For_i(0, reps) as _rep:
            emit_CAB("", 0, 1)
            emit_CAB("2", 1, 0)
    else:
        for r in range(reps):
            emit_CAB(f"{2 * r}", r % 2, 1 - r % 2)


def _build(reps=1, loop=True):
    """reps counts loop iterations; each iteration executes the computation
    twice (two pipelined body halves)."""
    key = (reps, loop)
    if key in _CACHE:
        return _CACHE[key]
    nc = bacc.Bacc("TRN2", target_bir_lowering=False, debug=False)
    d = {
        "xin": nc.dram_tensor("xin", [C, XIN], F16, kind="ExternalInput").ap(),
        "wb": nc.dram_tensor("wb", [3, 128, 30], F32, kind="ExternalInput").ap(),
        "wq": nc.dram_tensor("wq", [C, C], F16, kind="ExternalInput").ap(),
        "wk": nc.dram_tensor("wk", [C, C], F16, kind="ExternalInput").ap(),
        "wvp": nc.dram_tensor("wvp", [C, NH * 65], F16,
                              kind="ExternalInput").ap(),
        "wpj": nc.dram_tensor("wpj", [C, C], F16, kind="ExternalInput").ap(),
        "ind6": nc.dram_tensor("ind6", [3, 6, 128], F16,
                               kind="ExternalInput").ap(),
        "bpjW": nc.dram_tensor("bpjW", [128, 512], F32, kind="ExternalInput").ap(),
        "out": nc.dram_tensor("out", [T, C], F32, kind="ExternalOutput").ap(),
    }
    with tile.TileContext(nc) as tc:
        with contextlib.ExitStack() as ctx:
            _emit(nc, tc, ctx, d, reps, loop)
    nc.compile()
    _CACHE[key] = nc
    return nc


def _bpjw(bproj):
    w = np.zeros((128, 512), np.float32)
    w[:, 0:C] = bproj[None, :]
    return w


def _host_prep(x, conv_q, conv_k, conv_v, bn_q, bn_k, bn_v, Wq, Wk, Wv,
               Wproj, bproj):
    B = x.shape[0]
    x = np.asarray(x, np.float32)
    xt = np.ascontiguousarray(x.transpose(0, 2, 1))       # [B, C, T]
    ximg = xt.reshape(B, C, 56, 56)

    xin = np.zeros((B, C, XIN), np.float16)
    # xc: flat padded; xl/xr: column-shifted (zero at row seam)
    xin[:, :, 56:56 + T] = xt
    xl = ximg.copy(); xl[:, :, :, 1:] = ximg[:, :, :, :-1]; xl[:, :, :, 0] = 0
    xr = ximg.copy(); xr[:, :, :, :-1] = ximg[:, :, :, 1:]; xr[:, :, :, -1] = 0
    xin[:, :, XP + 56:XP + 56 + T] = xl.reshape(B, C, T)
    xin[:, :, 2 * XP + 56:2 * XP + 56 + T] = xr.reshape(B, C, T)
    # parity planes for the stride-2 convs: plane (p, dj) rows j=1+k hold
    # image row 2k+p, columns 2c+dj (zero-padded outside the image)
    xpad = np.zeros((B, C, 58, 58), np.float32)
    xpad[:, :, 1:57, 1:57] = ximg
    for par in range(2):
        for dj in (-1, 0, 1):
            pl = np.zeros((B, C, 30, 28), np.float32)
            rows = 1 + np.arange(28) * 2 + par
            cols = 1 + np.arange(28) * 2 + dj
            pl[:, :, 1:29, :] = xpad[:, :, rows[:, None], cols[None, :]]
            off = 3 * XP + (3 * par + dj + 1) * PLW
            xin[:, :, off:off + PLW] = pl.reshape(B, C, PLW)

    wb = np.zeros((3, 128, 30), np.float32)
    for cv, (w, bn) in enumerate(((conv_q, bn_q), (conv_k, bn_k),
                                  (conv_v, bn_v))):
        g, b, m, v = [np.asarray(bn[i], np.float64) for i in range(4)]
        a = g / np.sqrt(v + EPS)
        bias = (b - m * a).astype(np.float32)
        wh = (np.asarray(w, np.float64).reshape(C, 9) * a[:, None]).astype(
            np.float32)
        for ch in range(3):
            wb[ch, :, 9 * cv:9 * cv + 9] = wh[ch * 128:(ch + 1) * 128]
            wb[ch, :, 27 + cv] = bias[ch * 128:(ch + 1) * 128]

    wvp = np.zeros((C, NH * 65), np.float16)
    Wv = np.asarray(Wv, np.float32)
    for h in range(NH):
        wvp[:, h * 65:h * 65 + 64] = Wv[:, h * 64:(h + 1) * 64]

    ind6 = np.zeros((3, 6, 128), np.float16)
    for ch in range(3):
        ind6[ch, 2 * ch, 0:64] = 1.0
        ind6[ch, 2 * ch + 1, 64:128] = 1.0

    return {
        "xin": xin,
        "wb": wb,
        "wq": np.asarray(Wq, np.float16),
        "wk": np.asarray(Wk, np.float16),
        "wvp": wvp,
        "wpj": np.asarray(Wproj, np.float16),
        "ind6": ind6,
        "bpjW": _bpjw(np.asarray(bproj, np.float32)),
    }


def kernel(x, h, w, conv_q, conv_k, conv_v, bn_q, bn_k, bn_v, Wq, Wk, Wv,
           Wproj, bproj, _reps=1, _nc=None):
    B = x.shape[0]
    nc = _nc if _nc is not None else _build(_reps)
    hp = _host_prep(x, conv_q, conv_k, conv_v, bn_q, bn_k, bn_v, Wq, Wk, Wv,
                    Wproj, bproj)
    shared = {k: v for k, v in hp.items() if k != "xin"}
    in_maps = [dict(shared, xin=hp["xin"][b]) for b in range(B)]
    res = run_bass_kernel_spmd(nc, in_maps, core_ids=list(range(B)))
    out = np.stack([res.results[b]["out"] for b in range(B)], axis=0)
    return out.astype(np.float32)
